# revision 24
# baseline (speedup 1.0000x reference)
import sys

import numpy as np

for _p in ("/opt/trn_rl_repo",):
    if _p not in sys.path:
        sys.path.insert(0, _p)

import concourse.bass as bass
import concourse.mybir as mybir
from concourse import bacc
import concourse.tile as tile
from concourse import masks
from concourse.bass_utils import run_bass_kernel_spmd

B, N, E, H, DH = 64, 197, 768, 12, 64
NCORES = 8
BPC = B // NCORES  # batches per core
EPS = 1e-6
F32 = mybir.dt.float32
F16 = mybir.dt.float16
BF16 = mybir.dt.bfloat16

# token partition tiles (all 197 tokens incl cls)
TOK = ((0, 128), (128, 69))
GROUPS = BPC // 2  # 2 batches per group
GW = 2 * N  # 394
AF = mybir.ActivationFunctionType


def build_nc():
    nc = bacc.Bacc()
    # x pre-transposed on host: [group, E, bi, N] bf16 (one contiguous
    # [128, 394] DMA per 128-feature chunk per group)
    xt = nc.declare_dram_parameter("xt", [GROUPS, E, 2, N], BF16, isOutput=False)
    wq = nc.declare_dram_parameter("wq", [E, E], BF16, isOutput=False)
    wk = nc.declare_dram_parameter("wk", [E, E], BF16, isOutput=False)
    wv = nc.declare_dram_parameter("wv", [E, E], BF16, isOutput=False)
    wva = nc.declare_dram_parameter("wva", [E, 36], BF16, isOutput=False)
    # l6[h] = L6 block at rows 6h..6h+5, zeros elsewhere (K=72 lhsT variants,
    # sidesteps the PE base-partition-must-be-0/32/64 rule)
    l6 = nc.declare_dram_parameter("l6", [H, 72, N], F16, isOutput=False)
    p2 = nc.declare_dram_parameter("p2", [N, 4], F32, isOutput=False)
    bias3 = nc.declare_dram_parameter("bias3", [128, 36], F32, isOutput=False)
    outc = nc.declare_dram_parameter("outc", [BPC, N, E], F32, isOutput=True)

    with tile.TileContext(nc) as tc:
        from contextlib import ExitStack

        with ExitStack() as ctx:
            ep = ctx.enter_context

            cpool = ep(tc.tile_pool(name="const", bufs=1))
            xTpool = ep(tc.tile_pool(name="xT", bufs=2))
            qkpool = ep(tc.tile_pool(name="qk", bufs=2))
            vpool = ep(tc.tile_pool(name="v", bufs=2))
            spool = ep(tc.tile_pool(name="small", bufs=2))
            rpool = ep(tc.tile_pool(name="r", bufs=4))
            btpool = ep(tc.tile_pool(name="bt", bufs=2))
            epool = ep(tc.tile_pool(name="e", bufs=3))
            opool = ep(tc.tile_pool(name="out", bufs=2))

            # PSUM banks: big 2 + arg 2 + av 2x2 = 8
            ps_big = ep(tc.tile_pool(name="ps_big", bufs=2, space="PSUM"))
            ps_arg = ep(tc.tile_pool(name="ps_arg", bufs=2, space="PSUM"))
            ps_av = ep(tc.tile_pool(name="ps_av", bufs=2, space="PSUM"))

            # ---- constants ----
            identb = cpool.tile([128, 128], BF16, tag="identb")
            masks.make_identity(nc, identb[:, :])
            nc.vector.tensor_scalar_add(identb[:, :], identb[:, :], 0.0)
            identh = cpool.tile([128, 128], F16, tag="identh")
            masks.make_identity(nc, identh[:, :])
            nc.vector.tensor_scalar_add(identh[:, :], identh[:, :], 0.0)

            def emit_xt_dma(g, st):
                """DMA pre-transposed x chunks straight into xT tiles."""
                for eb in range(6):
                    t = xTpool.tile([128, GW], BF16, tag=f"xT{eb}", name=f"xT{eb}")
                    nc.gpsimd.dma_start(
                        t[:, :].rearrange("p (b n) -> p b n", n=N),
                        xt[g, eb * 128 : (eb + 1) * 128, :, :],
                    )
                    st["xT"].append(t)

            st0 = {
                "xT": [], "q": [], "k": [], "R": [],
                "v": [[None, None], [None, None]], "bt": {},
            }
            # wq first (q matmuls are the first real PE work), then x
            w_raw = {}
            for name, dram in (("q", wq),):
                raw = cpool.tile([128, 6 * E], BF16, tag=f"wr{name}", name=f"wr{name}")
                nc.gpsimd.dma_start(
                    raw[:, :].rearrange("p (ke f) -> p ke f", f=E),
                    dram.rearrange("(ke p) f -> p ke f", p=128),
                )
                w_raw[name] = raw
            emit_xt_dma(0, st0)

            # PE warm-up: ~5us of dummy matmuls while weights stream in, so
            # HAM un-throttles (K=8/8) before the real q/k projections start
            warm_sb = cpool.tile([128, 512], BF16, tag="warm_sb")
            nc.vector.memset(warm_sb[:, :], 0.0)
            warm_ps = ps_big.tile([128, 512], F32, tag="big", name="warm")
            for wi in range(12):
                nc.tensor.matmul(
                    warm_ps[:, :],
                    identb[:, :],
                    warm_sb[:, :],
                    start=(wi == 0),
                    stop=(wi == 11),
                )

            # remaining weights: one DMA per matrix into a [128, 6*768] tile
            # (ke chunk at cols ke*768); consumed directly (no DVE staging)
            for name, dram in (("k", wk), ("v", wv)):
                raw = cpool.tile([128, 6 * E], BF16, tag=f"wr{name}", name=f"wr{name}")
                nc.gpsimd.dma_start(
                    raw[:, :].rearrange("p (ke f) -> p ke f", f=E),
                    dram.rearrange("(ke p) f -> p ke f", p=128),
                )
                w_raw[name] = raw
            w_big = w_raw
            wva_t = cpool.tile([128, 6 * 36], BF16, tag="wvar")
            nc.gpsimd.dma_start(
                wva_t[:, :].rearrange("p (ke f) -> p ke f", f=36),
                wva.rearrange("(ke p) f -> p ke f", p=128),
            )
            l6_t = cpool.tile([72, H * N], F16, tag="l6r")
            nc.gpsimd.dma_start(
                l6_t[:, :].rearrange("p (h n) -> p h n", n=N),
                l6.rearrange("h p n -> p h n"),
            )
            p2_t = []
            for tt, (toff, tcnt) in enumerate(TOK):
                t = cpool.tile([128, 4], F32, tag=f"p2{tt}")
                nc.gpsimd.dma_start(t[:tcnt, :], p2[toff : toff + tcnt, :])
                p2_t.append(t)
            bias_t = cpool.tile([128, 36], F32, tag="bias3")
            nc.gpsimd.dma_start(bias_t[:, :], bias3[:, :])

            def prep_blocks(g, st):
                """Generator: q/k projection PE blocks for group g.

                Yields after each PSUM-allocating block so the caller can
                interleave these dense chains into the previous group's
                attention stream (keeps PE activity high -> HAM stays warm).
                """
                xT = st["xT"]
                for nm in ("q", "k"):
                    wb = w_big[nm]
                    for mo in range(6):
                        ps = ps_big.tile([128, GW], F32, tag="big", name="psqk")
                        for ke in range(6):
                            nc.tensor.matmul(
                                ps[:, :],
                                wb[:, ke * E + mo * 128 : ke * E + (mo + 1) * 128],
                                xT[ke][:, :],
                                start=(ke == 0),
                                stop=(ke == 5),
                            )
                        t = qkpool.tile(
                            [128, GW], BF16, tag=f"{nm}T{mo}", name=f"{nm}T{mo}"
                        )
                        nc.vector.tensor_scalar_add(t[:, :], ps[:, :], 0.0)
                        st[nm].append(t)
                        yield

            def gauss_blocks(st):
                # --- gaussian params -> R_T[bi] [72, 197] f16 (rows 6h+k) ---
                # softplus and ln(softplus) as DVE polynomials (inputs stay in
                # [-0.55, 0.55]; fits are exact to ~2e-4 over [-0.8, 0.8]) so
                # the scalar engine only ever runs Exp -> no act-table reloads.
                ALU = mybir.AluOpType
                qTb = st["q"]
                R_T = st["R"]
                for bi in range(2):
                    rtps = ps_arg.tile([72, N], F16, tag="arg", name="rtps")
                    for pt, (poff, pcnt) in enumerate(TOK):
                        p36 = ps_arg.tile([128, 36], F32, tag="arg", name="p36")
                        for ke in range(6):
                            nc.tensor.matmul(
                                p36[:pcnt, :],
                                qTb[ke][:, bi * N + poff : bi * N + poff + pcnt],
                                wva_t[:, ke * 36 : (ke + 1) * 36],
                                start=(ke == 0),
                                stop=(ke == 5),
                            )
                        spa = spool.tile([128, 36], F32, tag="spa")
                        nc.vector.tensor_add(spa[:pcnt, :], p36[:pcnt, :], bias_t[:pcnt, :])
                        t2 = spool.tile([128, 36], F32, tag="t2")
                        nc.vector.tensor_mul(t2[:pcnt, :], spa[:pcnt, :], spa[:pcnt, :])
                        sp3 = spa[:pcnt, :].rearrange("p (h c) -> p h c", c=3)
                        t23 = t2[:pcnt, :].rearrange("p (h c) -> p h c", c=3)
                        # softplus(v) ~ (c4*t2 + c2)*t2 + 0.5*v + ln2 on var cols
                        w24 = spool.tile([128, 24], F32, tag="w24")
                        w242 = w24[:pcnt, :].rearrange("p (h c) -> p h c", c=2)
                        nc.vector.tensor_scalar(
                            w242, t23[:, :, 0:2], -0.00492024, 0.12493955,
                            ALU.mult, ALU.add,
                        )
                        nc.vector.tensor_mul(w242, w242, t23[:, :, 0:2])
                        s24 = spool.tile([128, 24], F32, tag="s24")
                        s242 = s24[:pcnt, :].rearrange("p (h c) -> p h c", c=2)
                        nc.vector.tensor_scalar(
                            s242, sp3[:, :, 0:2], 0.5, 0.69314901, ALU.mult, ALU.add
                        )
                        # rv = 1/(softplus + 2eps)
                        rv = spool.tile([128, 24], F32, tag="rv")
                        nc.vector.tensor_add(rv[:pcnt, :], w24[:pcnt, :], s24[:pcnt, :])
                        nc.vector.tensor_scalar_add(rv[:pcnt, :], rv[:pcnt, :], 2.0 * EPS)
                        nc.vector.reciprocal(rv[:pcnt, :], rv[:pcnt, :])
                        rv3 = rv[:pcnt, :].rearrange("p (h c) -> p h c", c=2)
                        rvx = rv3[:, :, 0:1]
                        rvy = rv3[:, :, 1:2]
                        # ln(softplus(a)) ~ ((c3*a + c2)*a + c1)*a + c0 on alpha col
                        aview = sp3[:, :, 2:3]
                        lna = spool.tile([128, 12], F32, tag="lna")
                        lnav = lna[:pcnt, :].unsqueeze(2)
                        nc.vector.tensor_scalar(
                            lnav, aview, -0.00479690, -0.07857014, ALU.mult, ALU.add
                        )
                        nc.vector.tensor_mul(lnav, lnav, aview)
                        nc.vector.tensor_scalar_add(lnav, lnav, 0.72132411)
                        nc.vector.tensor_mul(lnav, lnav, aview)
                        nc.vector.tensor_scalar_add(lnav, lnav, -0.36659306)
                        # R rows per head: [lna-0.5(rvx*px^2+rvy*py^2), rvx*px,
                        #                   -0.5rvx, rvy*py, -0.5rvy, -40]
                        px = p2_t[pt][:pcnt, 0:1]
                        px2 = p2_t[pt][:pcnt, 1:2]
                        py = p2_t[pt][:pcnt, 2:3]
                        py2 = p2_t[pt][:pcnt, 3:4]
                        rpre = rpool.tile([128, 72], F16, tag="rpre")
                        r6 = rpre[:pcnt, :].rearrange("p (h k) -> p h k", k=6)
                        nc.vector.tensor_scalar_mul(r6[:, :, 1:2], rvx, px)
                        nc.vector.tensor_scalar_mul(r6[:, :, 3:4], rvy, py)
                        nc.vector.tensor_scalar_mul(r6[:, :, 2:3], rvx, -0.5)
                        nc.vector.tensor_scalar_mul(r6[:, :, 4:5], rvy, -0.5)
                        ta = spool.tile([128, 12], F32, tag="ta")
                        tb2 = spool.tile([128, 12], F32, tag="tb2")
                        nc.vector.tensor_scalar_mul(ta[:pcnt, :].unsqueeze(2), rvx, px2)
                        nc.vector.tensor_scalar_mul(tb2[:pcnt, :].unsqueeze(2), rvy, py2)
                        tc2 = spool.tile([128, 12], F32, tag="tc2")
                        nc.vector.tensor_add(tc2[:pcnt, :], ta[:pcnt, :], tb2[:pcnt, :])
                        nc.vector.tensor_scalar_mul(tc2[:pcnt, :], tc2[:pcnt, :], -0.5)
                        nc.vector.tensor_add(
                            r6[:, :, 0:1],
                            tc2[:pcnt, :].unsqueeze(2),
                            lna[:pcnt, :].unsqueeze(2),
                        )
                        nc.vector.memset(r6[:, :, 5:6], -40.0)
                        if pt == 0:
                            # cls query col: zero linear terms, force R0 (and keep
                            # R5) at -40 so bias underflows to 0 for i=0 and (0,0)
                            r60 = rpre[0:1, :].rearrange("p (h k) -> p h k", k=6)
                            nc.vector.memset(r60[:, :, 0:5], 0.0)
                            nc.vector.memset(r60[:, :, 0:1], -40.0)
                        nc.tensor.matmul(
                            rtps[:72, poff : poff + pcnt],
                            rpre[:pcnt, :72],
                            identh[:pcnt, :pcnt],
                            is_transpose=True,
                            start=(pt == 0),
                            stop=(pt == 1),
                        )
                        yield
                    t = rpool.tile([72, N], F16, tag="rT", name="rT")
                    nc.vector.tensor_scalar_add(t[:, :], rtps[:, :], 0.0)
                    R_T.append(t)
                    yield

            def qk_gauss_chain(g, st):
                """q blocks, then k blocks zipped with the gaussian DVE chain
                (k matmuls keep PE dense while DVE crunches softplus polys)."""
                pq = prep_blocks(g, st)
                for _ in range(6):  # q blocks
                    next(pq)
                    yield
                gz = gauss_blocks(st)
                while True:
                    a = next(pq, StopIteration)
                    if a is not StopIteration:
                        yield
                    b = next(gz, StopIteration)
                    if b is not StopIteration:
                        yield
                    if a is StopIteration and b is StopIteration:
                        return

            for _ in qk_gauss_chain(0, st0):
                pass
            states = {0: st0}

            # --- v projection and bias-tile generators (per batch) ---
            def v_blocks(vst, bi):
                xT = vst["xT"]
                for tb, (toff, tcnt) in enumerate(TOK):
                    t = vpool.tile(
                        [128, H * 65], BF16, tag=f"v{bi}{tb}", name=f"v{bi}{tb}"
                    )
                    tv = t[:tcnt, :].rearrange("p (h c) -> p h c", c=65)
                    for nb in range(2):
                        ps = ps_arg.tile([128, 384], F32, tag="arg", name="psv")
                        for ke in range(6):
                            nc.tensor.matmul(
                                ps[:tcnt, :],
                                xT[ke][:, bi * N + toff : bi * N + toff + tcnt],
                                w_big["v"][
                                    :, ke * E + nb * 384 : ke * E + (nb + 1) * 384
                                ],
                                start=(ke == 0),
                                stop=(ke == 5),
                            )
                        nc.vector.tensor_scalar_add(
                            tv[:, nb * 6 : (nb + 1) * 6, 0:64],
                            ps[:tcnt, :].rearrange("p (h c) -> p h c", c=64),
                            0.0,
                        )
                    nc.vector.memset(tv[:, :, 64:65], 1.0)
                    vst["v"][bi][tb] = t
                    yield

            def bt_blocks(vst, bi):
                # bias tiles: exp of the rank-6 arg matmul
                R_T = vst["R"]
                for pg in range(2):
                    for pk in range(3):
                        h0 = 4 * pk + pg
                        for jt, (joff, jcnt) in enumerate(TOK):
                            pa = ps_arg.tile([128, GW], F32, tag="arg", name="psarg")
                            for hh in range(2):
                                h = h0 + 2 * hh
                                nc.tensor.matmul(
                                    pa[:jcnt, hh * N : (hh + 1) * N],
                                    l6_t[:, h * N + joff : h * N + joff + jcnt],
                                    R_T[bi][:, :],
                                    start=(hh == 0),
                                    stop=(hh == 1),
                                )
                            bt = btpool.tile(
                                [128, GW], BF16, tag=f"bt{bi}{pg}{pk}{jt}", name="bt"
                            )
                            nc.scalar.activation(bt[:jcnt, :], pa[:jcnt, :], AF.Exp)
                            vst["bt"][bi, pg, pk, jt] = bt
                            yield

            # ---- main loop over 2-batch groups ----
            for g in range(GROUPS):
                st = states[g]
                xT, qTb, kTb = st["xT"], st["q"], st["k"]
                v_sb, bt_t = st["v"], st["bt"]

                if g == 0:
                    # group 0's v0/bt0 run up front; later groups get them via
                    # the previous group's fill chain
                    for _ in v_blocks(st, 0):
                        pass
                    for _ in bt_blocks(st, 0):
                        pass

                # fill chain interleaved into the attention streams: this
                # group's batch-1 v/bias, then next group's q/k + gaussian
                # (hoisted so PE never waits on the DVE chain) and next
                # group's batch-0 v/bias (spreads the ACT-bound exp bursts)
                from itertools import chain as _chain

                if g + 1 < GROUPS:
                    st1 = {
                        "xT": [], "q": [], "k": [], "R": [],
                        "v": [[None, None], [None, None]], "bt": {},
                    }
                    states[g + 1] = st1
                    emit_xt_dma(g + 1, st1)
                    fill_gen = _chain(
                        v_blocks(st, 1),
                        bt_blocks(st, 1),
                        qk_gauss_chain(g + 1, st1),
                        v_blocks(st1, 0),
                        bt_blocks(st1, 0),
                    )
                else:
                    fill_gen = _chain(v_blocks(st, 1), bt_blocks(st, 1))

                def interleave():
                    next(fill_gen, None)

                # --- attention: same-parity head pairs (h, h+2) so both heads
                # share lhsT base partitions -> one PSUM bank per pair ---
                out_sb = [
                    [
                        opool.tile([128, E], F32, tag=f"o{bi}{it}", name=f"o{bi}{it}")
                        for it in range(2)
                    ]
                    for bi in range(2)
                ]
                for bi in range(2):
                    for pg in range(2):
                        ro = 64 * pg
                        av = [
                            ps_av.tile([128, 6 * 65], F32, tag=f"av{it}", name=f"av{it}")
                            for it in range(2)
                        ]

                        def av_block(pk, e_t):
                            h0 = 4 * pk + pg
                            for it, (ioff, icnt) in enumerate(TOK):
                                for hh in range(2):
                                    h = h0 + 2 * hh
                                    col = (2 * pk + hh) * 65
                                    for jt, (joff, jcnt) in enumerate(TOK):
                                        nc.tensor.matmul(
                                            av[it][:icnt, col : col + 65],
                                            e_t[jt][
                                                :jcnt, hh * N + ioff : hh * N + ioff + icnt
                                            ],
                                            v_sb[bi][jt][:jcnt, h * 65 : h * 65 + 65],
                                            start=(pk == 0 and hh == 0 and jt == 0),
                                            stop=(pk == 2 and hh == 1 and jt == 1),
                                        )

                        prev = None
                        for pk in range(4):  # 3 pairs + AV lagged one pair
                            if pk < 3:
                                h0 = 4 * pk + pg
                                e_t = []
                                for jt, (joff, jcnt) in enumerate(TOK):
                                    ps = ps_big.tile([128, GW], F32, tag="big", name="pssc")
                                    for hh in range(2):
                                        h = h0 + 2 * hh
                                        mo = h // 2
                                        nc.tensor.matmul(
                                            ps[:jcnt, hh * N : (hh + 1) * N],
                                            kTb[mo][
                                                ro : ro + 64,
                                                bi * N + joff : bi * N + joff + jcnt,
                                            ],
                                            qTb[mo][ro : ro + 64, bi * N : bi * N + N],
                                            start=(hh == 0),
                                            stop=False,
                                        )
                                    nc.tensor.matmul(
                                        ps[:jcnt, :],
                                        identb[:jcnt, :jcnt],
                                        bt_t[bi, pg, pk, jt][:jcnt, :],
                                        start=False,
                                        stop=True,
                                    )
                                    e = epool.tile(
                                        [128, GW], BF16, tag=f"e{jt}", name=f"e{jt}"
                                    )
                                    nc.scalar.activation(e[:jcnt, :], ps[:jcnt, :], AF.Exp)
                                    e_t.append(e)
                                    interleave()
                            if pk >= 1:
                                av_block(*prev)
                                interleave()
                            prev = (pk, e_t) if pk < 3 else None
                        # normalize 6 heads at once per token tile
                        for it, (ioff, icnt) in enumerate(TOK):
                            av3 = av[it][:icnt, :].rearrange("p (h c) -> p h c", c=65)
                            rr = spool.tile([128, 6], F32, tag="rr")
                            nc.vector.reciprocal(rr[:icnt, :].unsqueeze(2), av3[:, :, 64:65])
                            ov = out_sb[bi][it][:icnt, :].rearrange(
                                "p (k two d) -> p k two d", two=2, d=64
                            )[:, :, pg, :]
                            nc.vector.tensor_mul(
                                ov,
                                av3[:, :, 0:64],
                                rr[:icnt, :].unsqueeze(2).broadcast_to([icnt, 6, 64]),
                            )
                for bi in range(2):
                    for it, (toff, tcnt) in enumerate(TOK):
                        nc.gpsimd.dma_start(
                            outc[2 * g + bi, toff : toff + tcnt, :],
                            out_sb[bi][it][:tcnt, :],
                        )
                # flush any remaining fill blocks
                for _ in fill_gen:
                    pass
    nc.compile()
    return nc


_NC_CACHE = None


def _get_nc():
    global _NC_CACHE
    if _NC_CACHE is None:
        _NC_CACHE = build_nc()
    return _NC_CACHE


def _prep_inputs(x, Wq, Wk, Wv, W_var, b_var, W_alpha, b_alpha, diff):
    import ml_dtypes

    bf16 = ml_dtypes.bfloat16
    x = np.asarray(x, np.float32)
    wq = np.ascontiguousarray(np.asarray(Wq, np.float32).T).astype(bf16)
    wk = np.ascontiguousarray(np.asarray(Wk, np.float32).T * 0.125).astype(bf16)
    wv = np.ascontiguousarray(np.asarray(Wv, np.float32).T).astype(bf16)
    W_var = np.asarray(W_var, np.float32)
    W_alpha = np.asarray(W_alpha, np.float32)
    diff = np.asarray(diff)
    # block-diagonal [768, 36]: cols 3h+{0,1,2} = W_var[0], W_var[1], W_alpha
    wva = np.zeros((E, 36), np.float32)
    for h in range(H):
        sl = slice(h * DH, (h + 1) * DH)
        wva[sl, 3 * h + 0] = W_var[0]
        wva[sl, 3 * h + 1] = W_var[1]
        wva[sl, 3 * h + 2] = W_alpha[0]
    wva = wva.astype(bf16)
    # grid coordinates per token (derived from diff against patch 0 at (0,0))
    pxp = np.sqrt(diff[:, 0, 0].astype(np.float64)).astype(np.float32)  # (196,)
    pyp = np.sqrt(diff[:, 0, 1].astype(np.float64)).astype(np.float32)
    px = np.concatenate([[0.0], pxp]).astype(np.float32)  # (197,) token-indexed
    py = np.concatenate([[0.0], pyp]).astype(np.float32)
    # L6 [6, 197]: col j>=1 -> [1, px, px^2, py, py^2, 0]; col 0 (cls) -> e_5
    l6a = np.zeros((6, N), np.float32)
    l6a[0, 1:] = 1.0
    l6a[1, 1:] = px[1:]
    l6a[2, 1:] = px[1:] ** 2
    l6a[3, 1:] = py[1:]
    l6a[4, 1:] = py[1:] ** 2
    l6a[5, 0] = 1.0
    # 12 block lhsT variants: l6[h] has L6 at rows 6h..6h+5, zeros elsewhere
    l6 = np.zeros((H, 72, N), np.float32)
    for h in range(H):
        l6[h, 6 * h : 6 * h + 6] = l6a
    l6 = l6.astype(np.float16)
    p2 = np.stack([px, px**2, py, py**2], axis=1).astype(np.float32)  # (197, 4)
    bias3 = np.tile(
        np.concatenate([np.asarray(b_var, np.float32), np.asarray(b_alpha, np.float32)]),
        (128, H),
    ).astype(np.float32)
    shared = dict(wq=wq, wk=wk, wv=wv, wva=wva, l6=l6, p2=p2, bias3=bias3)
    # pre-transpose x per core: [GROUPS, E, 2, N] bf16
    xb = x.astype(bf16)
    in_maps = []
    for c in range(NCORES):
        m = dict(shared)
        xc = xb[c * BPC : (c + 1) * BPC]  # [BPC, N, E]
        m["xt"] = np.ascontiguousarray(
            xc.reshape(BPC // 2, 2, N, E).transpose(0, 3, 1, 2)
        )
        in_maps.append(m)
    return in_maps


def run(trace=False, **inputs):
    nc = _get_nc()
    in_maps = _prep_inputs(**inputs)
    res = run_bass_kernel_spmd(nc, in_maps, list(range(NCORES)), trace=trace)
    out = np.concatenate([res.results[c]["outc"] for c in range(NCORES)], axis=0)
    return out, res


def kernel(**inputs):
    out, _ = run(trace=False, **inputs)
    return out



# revision 25
# speedup vs baseline: 1.1468x; 1.1468x over previous
import sys

import numpy as np

for _p in ("/opt/trn_rl_repo",):
    if _p not in sys.path:
        sys.path.insert(0, _p)

import concourse.bass as bass
import concourse.mybir as mybir
from concourse import bacc
import concourse.tile as tile
from concourse import masks
from concourse.bass_utils import run_bass_kernel_spmd

B, N, E, H, DH = 64, 197, 768, 12, 64
NCORES = 8
BPC = B // NCORES  # batches per core
EPS = 1e-6
F32 = mybir.dt.float32
F16 = mybir.dt.float16
BF16 = mybir.dt.bfloat16

# token partition tiles (all 197 tokens incl cls)
TOK = ((0, 128), (128, 69))
GROUPS = BPC // 2  # 2 batches per group
GW = 2 * N  # 394
AF = mybir.ActivationFunctionType


def build_nc():
    nc = bacc.Bacc()
    # x pre-transposed on host: [group, E, bi, N] bf16 (one contiguous
    # [128, 394] DMA per 128-feature chunk per group)
    xt = nc.declare_dram_parameter("xt", [GROUPS, E, 2, N], BF16, isOutput=False)
    wq = nc.declare_dram_parameter("wq", [E, E], BF16, isOutput=False)
    wk = nc.declare_dram_parameter("wk", [E, E], BF16, isOutput=False)
    wv = nc.declare_dram_parameter("wv", [E, E], BF16, isOutput=False)
    wva = nc.declare_dram_parameter("wva", [E, 36], BF16, isOutput=False)
    # l6[h] = L6 block at rows 6h..6h+5, zeros elsewhere (K=72 lhsT variants,
    # sidesteps the PE base-partition-must-be-0/32/64 rule)
    l6 = nc.declare_dram_parameter("l6", [H, 72, N], F16, isOutput=False)
    p2 = nc.declare_dram_parameter("p2", [N, 4], F32, isOutput=False)
    bias3 = nc.declare_dram_parameter("bias3", [128, 36], F32, isOutput=False)
    outc = nc.declare_dram_parameter("outc", [BPC, N, E], F32, isOutput=True)

    with tile.TileContext(nc) as tc:
        from contextlib import ExitStack

        with ExitStack() as ctx:
            ep = ctx.enter_context

            cpool = ep(tc.tile_pool(name="const", bufs=1))
            xTpool = ep(tc.tile_pool(name="xT", bufs=2))
            qkpool = ep(tc.tile_pool(name="qk", bufs=2))
            vpool = ep(tc.tile_pool(name="v", bufs=2))
            spool = ep(tc.tile_pool(name="small", bufs=2))
            rpool = ep(tc.tile_pool(name="r", bufs=4))
            btpool = ep(tc.tile_pool(name="bt", bufs=2))
            epool = ep(tc.tile_pool(name="e", bufs=3))
            opool = ep(tc.tile_pool(name="out", bufs=2))

            # PSUM banks: big 2 + arg 2 + av 2x2 = 8
            ps_big = ep(tc.tile_pool(name="ps_big", bufs=2, space="PSUM"))
            ps_arg = ep(tc.tile_pool(name="ps_arg", bufs=2, space="PSUM"))
            ps_av = ep(tc.tile_pool(name="ps_av", bufs=2, space="PSUM"))

            # ---- constants ----
            identb = cpool.tile([128, 128], BF16, tag="identb")
            masks.make_identity(nc, identb[:, :])
            nc.vector.tensor_scalar_add(identb[:, :], identb[:, :], 0.0)
            identh = cpool.tile([128, 128], F16, tag="identh")
            masks.make_identity(nc, identh[:, :])
            nc.vector.tensor_scalar_add(identh[:, :], identh[:, :], 0.0)

            def emit_xt_dma(g, st):
                """DMA pre-transposed x chunks straight into xT tiles."""
                for eb in range(6):
                    t = xTpool.tile([128, GW], BF16, tag=f"xT{eb}", name=f"xT{eb}")
                    nc.gpsimd.dma_start(
                        t[:, :].rearrange("p (b n) -> p b n", n=N),
                        xt[g, eb * 128 : (eb + 1) * 128, :, :],
                    )
                    st["xT"].append(t)

            st0 = {
                "xT": [], "q": [], "k": [], "R": [],
                "v": [[None, None], [None, None]], "bt": {},
            }
            # wq first (q matmuls are the first real PE work), then x
            w_raw = {}
            for name, dram in (("q", wq),):
                raw = cpool.tile([128, 6 * E], BF16, tag=f"wr{name}", name=f"wr{name}")
                nc.gpsimd.dma_start(
                    raw[:, :].rearrange("p (ke f) -> p ke f", f=E),
                    dram.rearrange("(ke p) f -> p ke f", p=128),
                )
                w_raw[name] = raw
            emit_xt_dma(0, st0)

            # PE warm-up: ~5us of dummy matmuls while weights stream in, so
            # HAM un-throttles (K=8/8) before the real q/k projections start
            warm_sb = cpool.tile([128, 512], BF16, tag="warm_sb")
            nc.vector.memset(warm_sb[:, :], 0.0)
            warm_ps = ps_big.tile([128, 512], F32, tag="big", name="warm")
            for wi in range(12):
                nc.tensor.matmul(
                    warm_ps[:, :],
                    identb[:, :],
                    warm_sb[:, :],
                    start=(wi == 0),
                    stop=(wi == 11),
                )

            # remaining weights: one DMA per matrix into a [128, 6*768] tile
            # (ke chunk at cols ke*768); consumed directly (no DVE staging)
            for name, dram in (("k", wk), ("v", wv)):
                raw = cpool.tile([128, 6 * E], BF16, tag=f"wr{name}", name=f"wr{name}")
                nc.gpsimd.dma_start(
                    raw[:, :].rearrange("p (ke f) -> p ke f", f=E),
                    dram.rearrange("(ke p) f -> p ke f", p=128),
                )
                w_raw[name] = raw
            w_big = w_raw
            wva_t = cpool.tile([128, 6 * 36], BF16, tag="wvar")
            nc.gpsimd.dma_start(
                wva_t[:, :].rearrange("p (ke f) -> p ke f", f=36),
                wva.rearrange("(ke p) f -> p ke f", p=128),
            )
            l6_t = cpool.tile([72, H * N], F16, tag="l6r")
            nc.gpsimd.dma_start(
                l6_t[:, :].rearrange("p (h n) -> p h n", n=N),
                l6.rearrange("h p n -> p h n"),
            )
            p2_t = []
            for tt, (toff, tcnt) in enumerate(TOK):
                t = cpool.tile([128, 4], F32, tag=f"p2{tt}")
                nc.gpsimd.dma_start(t[:tcnt, :], p2[toff : toff + tcnt, :])
                p2_t.append(t)
            bias_t = cpool.tile([128, 36], F32, tag="bias3")
            nc.gpsimd.dma_start(bias_t[:, :], bias3[:, :])

            def prep_blocks(g, st):
                """Generator: q/k projection PE blocks for group g.

                Yields after each PSUM-allocating block so the caller can
                interleave these dense chains into the previous group's
                attention stream (keeps PE activity high -> HAM stays warm).
                """
                xT = st["xT"]
                for nm in ("q", "k"):
                    wb = w_big[nm]
                    for mo in range(6):
                        ps = ps_big.tile([128, GW], F32, tag="big", name="psqk")
                        for ke in range(6):
                            nc.tensor.matmul(
                                ps[:, :],
                                wb[:, ke * E + mo * 128 : ke * E + (mo + 1) * 128],
                                xT[ke][:, :],
                                start=(ke == 0),
                                stop=(ke == 5),
                            )
                        t = qkpool.tile(
                            [128, GW], BF16, tag=f"{nm}T{mo}", name=f"{nm}T{mo}"
                        )
                        nc.vector.tensor_scalar_add(t[:, :], ps[:, :], 0.0)
                        st[nm].append(t)
                        yield

            def gauss_blocks(st):
                # --- gaussian params -> R_T[bi] [72, 197] f16 (rows 6h+k) ---
                # softplus and ln(softplus) as DVE polynomials (inputs stay in
                # [-0.55, 0.55]; fits are exact to ~2e-4 over [-0.8, 0.8]) so
                # the scalar engine only ever runs Exp -> no act-table reloads.
                ALU = mybir.AluOpType
                qTb = st["q"]
                R_T = st["R"]
                for bi in range(2):
                    rtps = ps_arg.tile([72, N], F16, tag="arg", name="rtps")
                    for pt, (poff, pcnt) in enumerate(TOK):
                        p36 = ps_arg.tile([128, 36], F32, tag="arg", name="p36")
                        for ke in range(6):
                            nc.tensor.matmul(
                                p36[:pcnt, :],
                                qTb[ke][:, bi * N + poff : bi * N + poff + pcnt],
                                wva_t[:, ke * 36 : (ke + 1) * 36],
                                start=(ke == 0),
                                stop=(ke == 5),
                            )
                        spa = spool.tile([128, 36], F32, tag="spa")
                        nc.vector.tensor_add(spa[:pcnt, :], p36[:pcnt, :], bias_t[:pcnt, :])
                        t2 = spool.tile([128, 36], F32, tag="t2")
                        nc.vector.tensor_mul(t2[:pcnt, :], spa[:pcnt, :], spa[:pcnt, :])
                        sp3 = spa[:pcnt, :].rearrange("p (h c) -> p h c", c=3)
                        t23 = t2[:pcnt, :].rearrange("p (h c) -> p h c", c=3)
                        # softplus(v) ~ (c4*t2 + c2)*t2 + 0.5*v + ln2 on var cols
                        w24 = spool.tile([128, 24], F32, tag="w24")
                        w242 = w24[:pcnt, :].rearrange("p (h c) -> p h c", c=2)
                        nc.vector.tensor_scalar(
                            w242, t23[:, :, 0:2], -0.00492024, 0.12493955,
                            ALU.mult, ALU.add,
                        )
                        nc.vector.tensor_mul(w242, w242, t23[:, :, 0:2])
                        s24 = spool.tile([128, 24], F32, tag="s24")
                        s242 = s24[:pcnt, :].rearrange("p (h c) -> p h c", c=2)
                        nc.vector.tensor_scalar(
                            s242, sp3[:, :, 0:2], 0.5, 0.69314901, ALU.mult, ALU.add
                        )
                        # rv = 1/(softplus + 2eps)
                        rv = spool.tile([128, 24], F32, tag="rv")
                        nc.vector.tensor_add(rv[:pcnt, :], w24[:pcnt, :], s24[:pcnt, :])
                        nc.vector.tensor_scalar_add(rv[:pcnt, :], rv[:pcnt, :], 2.0 * EPS)
                        nc.vector.reciprocal(rv[:pcnt, :], rv[:pcnt, :])
                        rv3 = rv[:pcnt, :].rearrange("p (h c) -> p h c", c=2)
                        rvx = rv3[:, :, 0:1]
                        rvy = rv3[:, :, 1:2]
                        # ln(softplus(a)) ~ ((c3*a + c2)*a + c1)*a + c0 on alpha col
                        aview = sp3[:, :, 2:3]
                        lna = spool.tile([128, 12], F32, tag="lna")
                        lnav = lna[:pcnt, :].unsqueeze(2)
                        nc.vector.tensor_scalar(
                            lnav, aview, -0.00479690, -0.07857014, ALU.mult, ALU.add
                        )
                        nc.vector.tensor_mul(lnav, lnav, aview)
                        nc.vector.tensor_scalar_add(lnav, lnav, 0.72132411)
                        nc.vector.tensor_mul(lnav, lnav, aview)
                        nc.vector.tensor_scalar_add(lnav, lnav, -0.36659306)
                        # R rows per head: [lna-0.5(rvx*px^2+rvy*py^2), rvx*px,
                        #                   -0.5rvx, rvy*py, -0.5rvy, -40]
                        px = p2_t[pt][:pcnt, 0:1]
                        px2 = p2_t[pt][:pcnt, 1:2]
                        py = p2_t[pt][:pcnt, 2:3]
                        py2 = p2_t[pt][:pcnt, 3:4]
                        rpre = rpool.tile([128, 72], F16, tag="rpre")
                        r6 = rpre[:pcnt, :].rearrange("p (h k) -> p h k", k=6)
                        nc.vector.tensor_scalar_mul(r6[:, :, 1:2], rvx, px)
                        nc.vector.tensor_scalar_mul(r6[:, :, 3:4], rvy, py)
                        nc.vector.tensor_scalar_mul(r6[:, :, 2:3], rvx, -0.5)
                        nc.vector.tensor_scalar_mul(r6[:, :, 4:5], rvy, -0.5)
                        ta = spool.tile([128, 12], F32, tag="ta")
                        tb2 = spool.tile([128, 12], F32, tag="tb2")
                        nc.vector.tensor_scalar_mul(ta[:pcnt, :].unsqueeze(2), rvx, px2)
                        nc.vector.tensor_scalar_mul(tb2[:pcnt, :].unsqueeze(2), rvy, py2)
                        tc2 = spool.tile([128, 12], F32, tag="tc2")
                        nc.vector.tensor_add(tc2[:pcnt, :], ta[:pcnt, :], tb2[:pcnt, :])
                        nc.vector.tensor_scalar_mul(tc2[:pcnt, :], tc2[:pcnt, :], -0.5)
                        nc.vector.tensor_add(
                            r6[:, :, 0:1],
                            tc2[:pcnt, :].unsqueeze(2),
                            lna[:pcnt, :].unsqueeze(2),
                        )
                        nc.vector.memset(r6[:, :, 5:6], -40.0)
                        if pt == 0:
                            # cls query col: zero linear terms, force R0 (and keep
                            # R5) at -40 so bias underflows to 0 for i=0 and (0,0)
                            r60 = rpre[0:1, :].rearrange("p (h k) -> p h k", k=6)
                            nc.vector.memset(r60[:, :, 0:5], 0.0)
                            nc.vector.memset(r60[:, :, 0:1], -40.0)
                        nc.tensor.matmul(
                            rtps[:72, poff : poff + pcnt],
                            rpre[:pcnt, :72],
                            identh[:pcnt, :pcnt],
                            is_transpose=True,
                            start=(pt == 0),
                            stop=(pt == 1),
                        )
                        yield
                    t = rpool.tile([72, N], F16, tag="rT", name="rT")
                    nc.vector.tensor_scalar_add(t[:, :], rtps[:, :], 0.0)
                    R_T.append(t)
                    yield

            def qk_gauss_chain(g, st):
                """q blocks, then k blocks zipped with the gaussian DVE chain
                (k matmuls keep PE dense while DVE crunches softplus polys)."""
                pq = prep_blocks(g, st)
                for _ in range(6):  # q blocks
                    next(pq)
                    yield
                gz = gauss_blocks(st)
                while True:
                    a = next(pq, StopIteration)
                    if a is not StopIteration:
                        yield
                    b = next(gz, StopIteration)
                    if b is not StopIteration:
                        yield
                    if a is StopIteration and b is StopIteration:
                        return

            for _ in qk_gauss_chain(0, st0):
                pass
            states = {0: st0}

            # --- v projection and bias-tile generators (per batch) ---
            def v_blocks(vst, bi):
                xT = vst["xT"]
                for tb, (toff, tcnt) in enumerate(TOK):
                    t = vpool.tile(
                        [128, H * 65], BF16, tag=f"v{bi}{tb}", name=f"v{bi}{tb}"
                    )
                    tv = t[:tcnt, :].rearrange("p (h c) -> p h c", c=65)
                    for nb in range(2):
                        ps = ps_arg.tile([128, 384], F32, tag="arg", name="psv")
                        for ke in range(6):
                            nc.tensor.matmul(
                                ps[:tcnt, :],
                                xT[ke][:, bi * N + toff : bi * N + toff + tcnt],
                                w_big["v"][
                                    :, ke * E + nb * 384 : ke * E + (nb + 1) * 384
                                ],
                                start=(ke == 0),
                                stop=(ke == 5),
                            )
                        nc.vector.tensor_scalar_add(
                            tv[:, nb * 6 : (nb + 1) * 6, 0:64],
                            ps[:tcnt, :].rearrange("p (h c) -> p h c", c=64),
                            0.0,
                        )
                    nc.vector.memset(tv[:, :, 64:65], 1.0)
                    vst["v"][bi][tb] = t
                    yield

            def bt_blocks(vst, bi):
                # bias tiles: exp of the rank-6 arg matmul
                R_T = vst["R"]
                for pg in range(2):
                    for pk in range(3):
                        h0 = 4 * pk + pg
                        for jt, (joff, jcnt) in enumerate(TOK):
                            pa = ps_arg.tile([128, GW], F32, tag="arg", name="psarg")
                            for hh in range(2):
                                h = h0 + 2 * hh
                                nc.tensor.matmul(
                                    pa[:jcnt, hh * N : (hh + 1) * N],
                                    l6_t[:, h * N + joff : h * N + joff + jcnt],
                                    R_T[bi][:, :],
                                    start=(hh == 0),
                                    stop=(hh == 1),
                                )
                            bt = btpool.tile(
                                [128, GW], BF16, tag=f"bt{bi}{pg}{pk}{jt}", name="bt"
                            )
                            nc.scalar.activation(bt[:jcnt, :], pa[:jcnt, :], AF.Exp)
                            vst["bt"][bi, pg, pk, jt] = bt
                            yield

            # ---- main loop over 2-batch groups ----
            for g in range(GROUPS):
                st = states[g]
                xT, qTb, kTb = st["xT"], st["q"], st["k"]
                v_sb, bt_t = st["v"], st["bt"]

                # batch 0's v and pg0 bias tiles run up front; the pg1 bias
                # tiles flow through the fill chain into attention bi0/pg0's
                # slots (halves the ACT-bound exp burst at group start)
                for _ in v_blocks(st, 0):
                    pass
                bt0 = bt_blocks(st, 0)
                for _ in range(6):
                    next(bt0)

                # fill chain interleaved into the attention streams; next
                # group's gaussian chain is hoisted here so PE never waits on
                # the DVE chain at group start
                from itertools import chain as _chain

                if g + 1 < GROUPS:
                    st1 = {
                        "xT": [], "q": [], "k": [], "R": [],
                        "v": [[None, None], [None, None]], "bt": {},
                    }
                    states[g + 1] = st1
                    emit_xt_dma(g + 1, st1)
                    fill_gen = _chain(
                        bt0,
                        v_blocks(st, 1),
                        bt_blocks(st, 1),
                        qk_gauss_chain(g + 1, st1),
                    )
                else:
                    fill_gen = _chain(bt0, v_blocks(st, 1), bt_blocks(st, 1))

                def interleave():
                    next(fill_gen, None)

                # --- attention: same-parity head pairs (h, h+2) so both heads
                # share lhsT base partitions -> one PSUM bank per pair ---
                out_sb = [
                    [
                        opool.tile([128, E], F32, tag=f"o{bi}{it}", name=f"o{bi}{it}")
                        for it in range(2)
                    ]
                    for bi in range(2)
                ]
                for bi in range(2):
                    for pg in range(2):
                        ro = 64 * pg
                        av = [
                            ps_av.tile([128, 6 * 65], F32, tag=f"av{it}", name=f"av{it}")
                            for it in range(2)
                        ]

                        def av_block(pk, e_t):
                            h0 = 4 * pk + pg
                            for it, (ioff, icnt) in enumerate(TOK):
                                for hh in range(2):
                                    h = h0 + 2 * hh
                                    col = (2 * pk + hh) * 65
                                    for jt, (joff, jcnt) in enumerate(TOK):
                                        nc.tensor.matmul(
                                            av[it][:icnt, col : col + 65],
                                            e_t[jt][
                                                :jcnt, hh * N + ioff : hh * N + ioff + icnt
                                            ],
                                            v_sb[bi][jt][:jcnt, h * 65 : h * 65 + 65],
                                            start=(pk == 0 and hh == 0 and jt == 0),
                                            stop=(pk == 2 and hh == 1 and jt == 1),
                                        )

                        prev = None
                        for pk in range(4):  # 3 pairs + AV lagged one pair
                            if pk < 3:
                                h0 = 4 * pk + pg
                                e_t = []
                                for jt, (joff, jcnt) in enumerate(TOK):
                                    ps = ps_big.tile([128, GW], F32, tag="big", name="pssc")
                                    for hh in range(2):
                                        h = h0 + 2 * hh
                                        mo = h // 2
                                        nc.tensor.matmul(
                                            ps[:jcnt, hh * N : (hh + 1) * N],
                                            kTb[mo][
                                                ro : ro + 64,
                                                bi * N + joff : bi * N + joff + jcnt,
                                            ],
                                            qTb[mo][ro : ro + 64, bi * N : bi * N + N],
                                            start=(hh == 0),
                                            stop=False,
                                        )
                                    nc.tensor.matmul(
                                        ps[:jcnt, :],
                                        identb[:jcnt, :jcnt],
                                        bt_t[bi, pg, pk, jt][:jcnt, :],
                                        start=False,
                                        stop=True,
                                    )
                                    e = epool.tile(
                                        [128, GW], BF16, tag=f"e{jt}", name=f"e{jt}"
                                    )
                                    nc.scalar.activation(e[:jcnt, :], ps[:jcnt, :], AF.Exp)
                                    e_t.append(e)
                                    interleave()
                            if pk >= 1:
                                av_block(*prev)
                                interleave()
                            prev = (pk, e_t) if pk < 3 else None
                        # normalize 6 heads at once per token tile
                        for it, (ioff, icnt) in enumerate(TOK):
                            av3 = av[it][:icnt, :].rearrange("p (h c) -> p h c", c=65)
                            rr = spool.tile([128, 6], F32, tag="rr")
                            nc.vector.reciprocal(rr[:icnt, :].unsqueeze(2), av3[:, :, 64:65])
                            ov = out_sb[bi][it][:icnt, :].rearrange(
                                "p (k two d) -> p k two d", two=2, d=64
                            )[:, :, pg, :]
                            nc.vector.tensor_mul(
                                ov,
                                av3[:, :, 0:64],
                                rr[:icnt, :].unsqueeze(2).broadcast_to([icnt, 6, 64]),
                            )
                for bi in range(2):
                    for it, (toff, tcnt) in enumerate(TOK):
                        nc.gpsimd.dma_start(
                            outc[2 * g + bi, toff : toff + tcnt, :],
                            out_sb[bi][it][:tcnt, :],
                        )
                # flush any remaining fill blocks
                for _ in fill_gen:
                    pass
    nc.compile()
    return nc


_NC_CACHE = None


def _get_nc():
    global _NC_CACHE
    if _NC_CACHE is None:
        _NC_CACHE = build_nc()
    return _NC_CACHE


def _prep_inputs(x, Wq, Wk, Wv, W_var, b_var, W_alpha, b_alpha, diff):
    import ml_dtypes

    bf16 = ml_dtypes.bfloat16
    x = np.asarray(x, np.float32)
    wq = np.ascontiguousarray(np.asarray(Wq, np.float32).T).astype(bf16)
    wk = np.ascontiguousarray(np.asarray(Wk, np.float32).T * 0.125).astype(bf16)
    wv = np.ascontiguousarray(np.asarray(Wv, np.float32).T).astype(bf16)
    W_var = np.asarray(W_var, np.float32)
    W_alpha = np.asarray(W_alpha, np.float32)
    diff = np.asarray(diff)
    # block-diagonal [768, 36]: cols 3h+{0,1,2} = W_var[0], W_var[1], W_alpha
    wva = np.zeros((E, 36), np.float32)
    for h in range(H):
        sl = slice(h * DH, (h + 1) * DH)
        wva[sl, 3 * h + 0] = W_var[0]
        wva[sl, 3 * h + 1] = W_var[1]
        wva[sl, 3 * h + 2] = W_alpha[0]
    wva = wva.astype(bf16)
    # grid coordinates per token (derived from diff against patch 0 at (0,0))
    pxp = np.sqrt(diff[:, 0, 0].astype(np.float64)).astype(np.float32)  # (196,)
    pyp = np.sqrt(diff[:, 0, 1].astype(np.float64)).astype(np.float32)
    px = np.concatenate([[0.0], pxp]).astype(np.float32)  # (197,) token-indexed
    py = np.concatenate([[0.0], pyp]).astype(np.float32)
    # L6 [6, 197]: col j>=1 -> [1, px, px^2, py, py^2, 0]; col 0 (cls) -> e_5
    l6a = np.zeros((6, N), np.float32)
    l6a[0, 1:] = 1.0
    l6a[1, 1:] = px[1:]
    l6a[2, 1:] = px[1:] ** 2
    l6a[3, 1:] = py[1:]
    l6a[4, 1:] = py[1:] ** 2
    l6a[5, 0] = 1.0
    # 12 block lhsT variants: l6[h] has L6 at rows 6h..6h+5, zeros elsewhere
    l6 = np.zeros((H, 72, N), np.float32)
    for h in range(H):
        l6[h, 6 * h : 6 * h + 6] = l6a
    l6 = l6.astype(np.float16)
    p2 = np.stack([px, px**2, py, py**2], axis=1).astype(np.float32)  # (197, 4)
    bias3 = np.tile(
        np.concatenate([np.asarray(b_var, np.float32), np.asarray(b_alpha, np.float32)]),
        (128, H),
    ).astype(np.float32)
    shared = dict(wq=wq, wk=wk, wv=wv, wva=wva, l6=l6, p2=p2, bias3=bias3)
    # pre-transpose x per core: [GROUPS, E, 2, N] bf16
    xb = x.astype(bf16)
    in_maps = []
    for c in range(NCORES):
        m = dict(shared)
        xc = xb[c * BPC : (c + 1) * BPC]  # [BPC, N, E]
        m["xt"] = np.ascontiguousarray(
            xc.reshape(BPC // 2, 2, N, E).transpose(0, 3, 1, 2)
        )
        in_maps.append(m)
    return in_maps


def run(trace=False, **inputs):
    nc = _get_nc()
    in_maps = _prep_inputs(**inputs)
    res = run_bass_kernel_spmd(nc, in_maps, list(range(NCORES)), trace=trace)
    out = np.concatenate([res.results[c]["outc"] for c in range(NCORES)], axis=0)
    return out, res


def kernel(**inputs):
    out, _ = run(trace=False, **inputs)
    return out

